# revision 6
# baseline (speedup 1.0000x reference)
import sys

sys.path.insert(0, "/opt/trn_rl_repo")

import numpy as np

# Problem constants (nn_Attention_34978213658826)
B, L, DM, NH, DH = 2, 2048, 1024, 16, 64
P = 128
LT = L // P            # 16 q/k tiles
MC = DM // P           # 8 m-chunks
G = 4                  # q-tiles per group for the z matmul
NG = LT // G
HPC = 4                # heads per core
NPAIR = 2              # head pairs per core
NEG = -1.0e30

_CACHE = {}


def _ts(i, n):
    return slice(i * n, (i + 1) * n)


def build_bass():
    import concourse.bass as bass
    import concourse.mybir as mybir
    import concourse.tile as tile
    from concourse import bacc

    f32 = mybir.dt.float32
    bf16 = mybir.dt.bfloat16
    AX = mybir.AxisListType
    AF = mybir.ActivationFunctionType

    nc = bacc.Bacc(None, target_bir_lowering=False)
    xT = nc.dram_tensor("xT", [DM, L], f32, kind="ExternalInput")
    wq = nc.dram_tensor("wq", [NPAIR, DM + 1, P], f32, kind="ExternalInput")
    wk = nc.dram_tensor("wk", [NPAIR, DM + 1, P], f32, kind="ExternalInput")
    wv = nc.dram_tensor("wv", [DM + 1, HPC * DH], f32, kind="ExternalInput")
    wo = nc.dram_tensor("wo", [NPAIR, P, DM], f32, kind="ExternalInput")
    msk = nc.dram_tensor("mask", [P, P], f32, kind="ExternalInput")
    idn = nc.dram_tensor("ident", [P, P], f32, kind="ExternalInput")
    out = nc.dram_tensor("out", [L, DM], f32, kind="ExternalOutput")

    with tile.TileContext(nc) as tc:
        with (
            tc.tile_pool(name="const", bufs=1) as const,
            tc.tile_pool(name="w", bufs=1) as wp,
            tc.tile_pool(name="qk", bufs=1) as qkp,
            tc.tile_pool(name="vz", bufs=1) as vzp,
        ):
            ident = const.tile([P, P], f32)
            nc.sync.dma_start(ident, idn[:, :])
            mask = const.tile([P, P], f32)
            nc.sync.dma_start(mask, msk[:, :])
            ones = const.tile([1, L], f32)
            nc.vector.memset(ones, 1.0)

            wq_t = wp.tile([P, NPAIR, MC, P], f32)
            wq_b = wp.tile([1, NPAIR, P], f32)
            wk_t = wp.tile([P, NPAIR, MC, P], f32)
            wk_b = wp.tile([1, NPAIR, P], f32)
            for _pr in range(NPAIR):
                nc.sync.dma_start(wq_t[:, _pr], wq[_pr, :DM, :].rearrange("(c p) h -> p c h", p=P))
                nc.sync.dma_start(wq_b[:, _pr], wq[_pr, DM : DM + 1, :])
                nc.sync.dma_start(wk_t[:, _pr], wk[_pr, :DM, :].rearrange("(c p) h -> p c h", p=P))
                nc.sync.dma_start(wk_b[:, _pr], wk[_pr, DM : DM + 1, :])
            wv_t = wp.tile([P, MC, HPC * DH], f32)
            nc.sync.dma_start(wv_t, wv[:DM, :].rearrange("(c p) h -> p c h", p=P))
            wv_b = wp.tile([1, HPC * DH], f32)
            nc.sync.dma_start(wv_b, wv[DM : DM + 1, :])
            wo_t = wp.tile([P, NPAIR, DM], f32)
            for _pr in range(NPAIR):
                nc.sync.dma_start(wo_t[:, _pr], wo[_pr, :, :])

            qT = qkp.tile([P, NPAIR, L], f32)
            kT = qkp.tile([P, NPAIR, L], f32)
            vv = vzp.tile([P, LT, HPC * DH], bf16)
            zst = [vzp.tile([P, NPAIR, G * P], f32, name=f"zst{g}", tag=f"zst{g}") for g in range(NG)]

            # ---------------- Stage A: projections ----------------
            with (
                tc.tile_pool(name="xt", bufs=1) as xtp,
                tc.tile_pool(name="proj_ps", bufs=4, space="PSUM") as proj_ps,
                tc.tile_pool(name="v_ps", bufs=2, space="PSUM") as v_ps,
            ):
                xt = xtp.tile([P, MC, L], f32)
                nc.sync.dma_start(xt, xT[:, :].rearrange("(c p) l -> p c l", p=P))

                NQ = L // 512
                for pr in range(NPAIR):
                    for wt, wb, dest, scale in (
                        (wq_t, wq_b, qT, 0.125),
                        (wk_t, wk_b, kT, 1.0),
                    ):
                        pss = [proj_ps.tile([P, 512], f32, name=f"pp{n}", tag="pp") for n in range(NQ)]
                        for m in range(MC):
                            for n in range(NQ):
                                nc.tensor.matmul(
                                    pss[n],
                                    lhsT=wt[:, pr, m, :],
                                    rhs=xt[:, m, _ts(n, 512)],
                                    start=(m == 0),
                                    stop=False,
                                )
                        for n in range(NQ):
                            nc.tensor.matmul(
                                pss[n],
                                lhsT=wb[:, pr, :],
                                rhs=ones[:, _ts(n, 512)],
                                start=False,
                                stop=True,
                            )
                            nc.scalar.mul(dest[:, pr, _ts(n, 512)], pss[n], scale)

                for lt in range(LT):
                    ps = v_ps.tile([P, HPC * DH], f32)
                    for m in range(MC):
                        nc.tensor.matmul(
                            ps,
                            lhsT=xt[:, m, _ts(lt, P)],
                            rhs=wv_t[:, m, :],
                            start=(m == 0),
                            stop=False,
                        )
                    nc.tensor.matmul(
                        ps, lhsT=ones[:, _ts(lt, P)], rhs=wv_b, start=False, stop=True
                    )
                    nc.scalar.copy(vv[:, lt, :], ps)

            # ---------------- Stage B: attention per head ----------------
            with (
                tc.tile_pool(name="s_ps", bufs=4, space="PSUM") as s_ps,
                tc.tile_pool(name="z_ps", bufs=2, space="PSUM") as z_ps,
                tc.tile_pool(name="o_ps", bufs=2, space="PSUM") as o_ps,
                tc.tile_pool(name="prow", bufs=3) as prowp,
                tc.tile_pool(name="pt", bufs=2) as ptp,
                tc.tile_pool(name="stat", bufs=6) as statp,
                tc.tile_pool(name="osb", bufs=3) as osbp,
            ):
                for pr in range(NPAIR):
                    for g in range(NG):
                        ptg = [ptp.tile([P, LT, G, P], bf16, name=f"ptg{h2}", tag=f"ptg{h2}") for h2 in range(2)]
                        for s in range(G):
                            i = g * G + s
                            klen = (i + 1) * P
                            nch = (klen + 511) // 512
                            sps2 = []
                            for h2 in range(2):
                                sps = []
                                for c in range(nch):
                                    cw = min(512, klen - c * 512)
                                    sp = s_ps.tile([P, 512], f32, tag="s")
                                    dlo = klen - P - c * 512  # diag block offset in chunk
                                    has_diag = 0 <= dlo < cw
                                    nc.tensor.matmul(
                                        sp[:, :cw],
                                        lhsT=qT[_ts(h2, DH), pr, _ts(i, P)],
                                        rhs=kT[_ts(h2, DH), pr, c * 512 : c * 512 + cw],
                                        start=True,
                                        stop=not has_diag,
                                    )
                                    if has_diag:
                                        nc.tensor.matmul(
                                            sp[:, dlo : dlo + P],
                                            lhsT=ident,
                                            rhs=mask,
                                            start=False,
                                            stop=True,
                                        )
                                    sps.append((sp, cw))
                                sps2.append(sps)
                            for h2 in range(2):
                                sps = sps2[h2]
                                mx = statp.tile([P, 4], f32, tag="mx")
                                for c, (sp, cw) in enumerate(sps):
                                    nc.vector.reduce_max(
                                        mx[:, c : c + 1], sp[:, :cw], axis=AX.X
                                    )
                                negm = statp.tile([P, 1], f32, tag="negm")
                                if nch > 1:
                                    m1 = statp.tile([P, 1], f32, tag="m1")
                                    nc.vector.reduce_max(m1, mx[:, :nch], axis=AX.X)
                                    nc.vector.tensor_scalar_mul(negm, m1, -1.0)
                                else:
                                    nc.vector.tensor_scalar_mul(negm, mx[:, :1], -1.0)
                                prow = prowp.tile([P, L], bf16)
                                sums = statp.tile([P, 4], f32, tag="sums")
                                for c, (sp, cw) in enumerate(sps):
                                    nc.scalar.activation(
                                        prow[:, c * 512 : c * 512 + cw],
                                        sp[:, :cw],
                                        AF.Exp,
                                        bias=negm,
                                        accum_out=sums[:, c : c + 1],
                                    )
                                stot = statp.tile([P, 1], f32, tag="stot")
                                if nch > 1:
                                    nc.vector.reduce_sum(stot, sums[:, :nch], axis=AX.X)
                                else:
                                    nc.vector.tensor_copy(stot, sums[:, :1])
                                sinv = statp.tile([P, 1], f32, tag="sinv")
                                nc.vector.reciprocal(sinv, stot)
                                nc.gpsimd.tensor_scalar_mul(
                                    prow[:, :klen], prow[:, :klen], sinv
                                )
                                nc.sync.dma_start_transpose(
                                    ptg[h2][:, : i + 1, s, :], prow[:, :klen]
                                )
                        # z matmuls for this group
                        for h2 in range(2):
                            hcol = (pr * 2 + h2) * DH
                            zps = z_ps.tile([DH, G * P], f32)
                            jmax = G * (g + 1)
                            for j in range(jmax):
                                sc = max(0, j - G * g)
                                nc.tensor.matmul(
                                    zps[:, sc * P :],
                                    lhsT=vv[:, j, hcol : hcol + DH],
                                    rhs=ptg[h2][:, j, sc:G, :],
                                    start=(j == 0),
                                    stop=(j == jmax - 1),
                                )
                            nc.scalar.copy(zst[g][_ts(h2, DH), pr, :], zps)

                # ---------------- Stage C: output projection ----------------
                for i in range(LT):
                    g, s = divmod(i, G)
                    osb = osbp.tile([P, DM], f32)
                    for mc2 in range(2):
                        ops = o_ps.tile([P, 512], f32)
                        for pr in range(NPAIR):
                            nc.tensor.matmul(
                                ops,
                                lhsT=zst[g][:, pr, _ts(s, P)],
                                rhs=wo_t[:, pr, _ts(mc2, 512)],
                                start=(pr == 0),
                                stop=(pr == 1),
                            )
                        nc.scalar.copy(osb[:, _ts(mc2, 512)], ops)
                    nc.sync.dma_start(out[_ts(i, P), :], osb)

    nc.finalize()
    return nc


def make_in_maps(normal_pre_resid, W_Q, W_K, W_V, W_O, b_Q, b_K, b_V, b_O):
    x = np.asarray(normal_pre_resid, np.float32)
    W_Q = np.asarray(W_Q, np.float32)
    W_K = np.asarray(W_K, np.float32)
    W_V = np.asarray(W_V, np.float32)
    W_O = np.asarray(W_O, np.float32)
    b_Q = np.asarray(b_Q, np.float32)
    b_K = np.asarray(b_K, np.float32)
    b_V = np.asarray(b_V, np.float32)

    mask = np.triu(np.full((P, P), NEG, np.float32), k=1)
    ident = np.eye(P, dtype=np.float32)
    in_maps = []
    for c in range(8):
        b, hg = divmod(c, 4)
        heads = [4 * hg + j for j in range(HPC)]
        xT = np.ascontiguousarray(x[b].T)  # [DM, L]

        def pack_qk(W, bias):
            prs = []
            for p_ in range(NPAIR):
                h0, h1 = heads[2 * p_], heads[2 * p_ + 1]
                wcat = np.concatenate([W[h0], W[h1]], axis=1)  # [DM, 128]
                bcat = np.concatenate([bias[h0], bias[h1]])[None, :]  # [1, 128]
                prs.append(np.concatenate([wcat, bcat], axis=0))  # [DM+1, 128]
            return np.ascontiguousarray(np.stack(prs))  # [2, DM+1, 128]

        wv_cat = np.concatenate([W_V[h] for h in heads], axis=1)  # [DM, 256]
        bv_cat = np.concatenate([b_V[h] for h in heads])[None, :]  # [1, 256]
        wv_full = np.ascontiguousarray(np.concatenate([wv_cat, bv_cat], axis=0))
        wo_prs = np.ascontiguousarray(
            np.stack(
                [
                    np.concatenate(
                        [W_O[heads[2 * p_]], W_O[heads[2 * p_ + 1]]], axis=0
                    )
                    for p_ in range(NPAIR)
                ]
            )
        )  # [2, 128, DM]

        in_maps.append(
            {
                "xT": xT,
                "wq": pack_qk(W_Q, b_Q),
                "wk": pack_qk(W_K, b_K),
                "wv": wv_full,
                "wo": wo_prs,
                "mask": mask,
                "ident": ident,
            }
        )
    return in_maps


def run_device(in_maps, **kwargs):
    from concourse.bass_utils import run_bass_kernel_spmd

    if "nc" not in _CACHE:
        _CACHE["nc"] = build_bass()
    return run_bass_kernel_spmd(_CACHE["nc"], in_maps, core_ids=list(range(8)), **kwargs)


def kernel(normal_pre_resid, W_Q, W_K, W_V, W_O, b_Q, b_K, b_V, b_O, **extra):
    b_O = np.asarray(b_O, np.float32)
    in_maps = make_in_maps(
        normal_pre_resid, W_Q, W_K, W_V, W_O, b_Q, b_K, b_V, b_O
    )
    res = run_device(in_maps)
    outs = [r["out"] for r in res.results]
    full = np.zeros((B, L, DM), np.float32)
    for c in range(8):
        full[c // 4] += outs[c]
    full += b_O[None, None, :]
    return full


# revision 10
# speedup vs baseline: 2.9635x; 2.9635x over previous
import sys

sys.path.insert(0, "/opt/trn_rl_repo")

import numpy as np

# Problem constants (nn_Attention_34978213658826)
B, L, DM, NH, DH = 2, 2048, 1024, 16, 64
P = 128
LT = L // P            # 16 q/k tiles
MC = DM // P           # 8 m-chunks
G = 4                  # q-tiles per group for the z matmul
NG = LT // G
HPC = 4                # heads per core
NPAIR = 2              # head pairs per core
NEG = -1.0e30
SCH = 1024             # scores psum chunk width

# fp32r (full-rate fp32 matmul, ~1.5e-4 rel err) per stage
FP32R_QK = True
FP32R_V = True
FP32R_OUT = True

_CACHE = {}


def _ts(i, n):
    return slice(i * n, (i + 1) * n)


def build_bass():
    import concourse.mybir as mybir
    import concourse.tile as tile
    from concourse import bacc

    f32 = mybir.dt.float32
    f32r = mybir.dt.float32r
    bf16 = mybir.dt.bfloat16
    AX = mybir.AxisListType
    AF = mybir.ActivationFunctionType

    def r(ap, on=True):
        return ap

    nc = bacc.Bacc(None, target_bir_lowering=False)
    xT = nc.dram_tensor("xT", [DM, L], f32, kind="ExternalInput")
    wq = nc.dram_tensor("wq", [NPAIR, DM + 1, P], f32, kind="ExternalInput")
    wk = nc.dram_tensor("wk", [NPAIR, DM + 1, P], f32, kind="ExternalInput")
    wv = nc.dram_tensor("wv", [DM + 1, HPC * DH], f32, kind="ExternalInput")
    wo = nc.dram_tensor("wo", [NPAIR, P, DM], f32, kind="ExternalInput")
    msk = nc.dram_tensor("mask", [P, P], bf16, kind="ExternalInput")
    idn = nc.dram_tensor("ident", [P, P], bf16, kind="ExternalInput")
    out = nc.dram_tensor("out", [L, DM], f32, kind="ExternalOutput")

    with tile.TileContext(nc) as tc:
        with (
            tc.tile_pool(name="const", bufs=1) as const,
            tc.tile_pool(name="w", bufs=1) as wp,
            tc.tile_pool(name="qk", bufs=1) as qkp,
            tc.tile_pool(name="vz", bufs=1) as vzp,
        ):
            ident = const.tile([P, P], bf16)
            nc.sync.dma_start(ident, idn[:, :])
            mask = const.tile([P, P], bf16)
            nc.sync.dma_start(mask, msk[:, :])
            ones_f = const.tile([1, L], f32)
            nc.vector.memset(ones_f, 1.0)
            if FP32R_QK or FP32R_V:
                ones = const.tile([1, L], f32r)
                nc.vector.tensor_copy(ones, ones_f)
            else:
                ones = ones_f

            wq_t = wp.tile([P, NPAIR, MC, P], f32r if FP32R_QK else f32)
            wq_b = wp.tile([1, NPAIR, P], f32r if FP32R_QK else f32)
            wk_t = wp.tile([P, NPAIR, MC, P], f32r if FP32R_QK else f32)
            wk_b = wp.tile([1, NPAIR, P], f32r if FP32R_QK else f32)
            for _pr in range(NPAIR):
                nc.gpsimd.dma_start(wq_t[:, _pr], wq[_pr, :DM, :].rearrange("(c p) h -> p c h", p=P))
                nc.gpsimd.dma_start(wq_b[:, _pr], wq[_pr, DM : DM + 1, :])
                nc.gpsimd.dma_start(wk_t[:, _pr], wk[_pr, :DM, :].rearrange("(c p) h -> p c h", p=P))
                nc.gpsimd.dma_start(wk_b[:, _pr], wk[_pr, DM : DM + 1, :])
            wv_t = wp.tile([P, MC, HPC * DH], f32r if FP32R_V else f32)
            nc.gpsimd.dma_start(wv_t, wv[:DM, :].rearrange("(c p) h -> p c h", p=P))
            wv_b = wp.tile([1, HPC * DH], f32r if FP32R_V else f32)
            nc.gpsimd.dma_start(wv_b, wv[DM : DM + 1, :])
            wo_t = wp.tile([P, NPAIR, DM], f32r if FP32R_OUT else f32)
            for _pr in range(NPAIR):
                nc.gpsimd.dma_start(wo_t[:, _pr], wo[_pr, :, :])

            qT = qkp.tile([P, NPAIR, L], f32)
            kT = qkp.tile([P, NPAIR, L], f32)
            vv = vzp.tile([P, LT, HPC * DH], bf16)
            zst = [vzp.tile([P, NPAIR, G * P], f32r if FP32R_OUT else f32, name=f"zst{g}", tag=f"zst{g}") for g in range(NG)]

            # ---------------- Stage A: projections ----------------
            with (
                tc.tile_pool(name="xt", bufs=1) as xtp,
                tc.tile_pool(name="proj_ps", bufs=4, space="PSUM") as proj_ps,
                tc.tile_pool(name="v_ps", bufs=2, space="PSUM") as v_ps,
            ):
                xt = xtp.tile([P, MC, L], f32r if (FP32R_QK and FP32R_V) else f32)
                nc.gpsimd.dma_start(xt, xT[:, :].rearrange("(c p) l -> p c l", p=P))

                NQ = L // 512
                for pr in range(NPAIR):
                    for wt, wb, dest, scale in (
                        (wq_t, wq_b, qT, 0.125),
                        (wk_t, wk_b, kT, 1.0),
                    ):
                        pss = [proj_ps.tile([P, 512], f32, name=f"pp{n}", tag="pp") for n in range(NQ)]
                        for m in range(MC):
                            for n in range(NQ):
                                nc.tensor.matmul(
                                    pss[n],
                                    lhsT=r(wt[:, pr, m, :], FP32R_QK),
                                    rhs=r(xt[:, m, _ts(n, 512)], FP32R_QK),
                                    start=(m == 0),
                                    stop=False,
                                )
                        for n in range(NQ):
                            nc.tensor.matmul(
                                pss[n],
                                lhsT=r(wb[:, pr, :], FP32R_QK),
                                rhs=r(ones[:, _ts(n, 512)], FP32R_QK),
                                start=False,
                                stop=True,
                            )
                            nc.scalar.mul(dest[:, pr, _ts(n, 512)], pss[n], scale)

                for lt in range(LT):
                    ps = v_ps.tile([P, HPC * DH], f32)
                    for m in range(MC):
                        nc.tensor.matmul(
                            ps,
                            lhsT=r(xt[:, m, _ts(lt, P)], FP32R_V),
                            rhs=r(wv_t[:, m, :], FP32R_V),
                            start=(m == 0),
                            stop=False,
                        )
                    nc.tensor.matmul(
                        ps,
                        lhsT=r(ones[:, _ts(lt, P)], FP32R_V),
                        rhs=r(wv_b, FP32R_V),
                        start=False,
                        stop=True,
                    )
                    nc.scalar.copy(vv[:, lt, :], ps)

            # ---------------- Stage B: attention per head ----------------
            with (
                tc.tile_pool(name="s_ps", bufs=3, space="PSUM") as s_ps,
                tc.tile_pool(name="z_ps", bufs=1, space="PSUM") as z_ps,
                tc.tile_pool(name="o_ps", bufs=1, space="PSUM") as o_ps,
                tc.tile_pool(name="prow", bufs=3) as prowp,
                tc.tile_pool(name="pt", bufs=2) as ptp,
                tc.tile_pool(name="stat", bufs=6) as statp,
                tc.tile_pool(name="osb", bufs=2) as osbp,
            ):
                for pr in range(NPAIR):
                    for g in range(NG):
                        ptg = [ptp.tile([P, LT, G, P], bf16, name=f"ptg{h2}", tag=f"ptg{h2}") for h2 in range(2)]
                        for s in range(G):
                            i = g * G + s
                            klen = (i + 1) * P
                            nch = (klen + SCH - 1) // SCH
                            sps2 = [[], []]
                            # interleave the two heads' chunk matmuls (K=64
                            # row-tiled pairs run concurrently on the PE)
                            for c in range(nch):
                                cw = min(SCH, klen - c * SCH)
                                dlo = klen - P - c * SCH  # diag block offset
                                has_diag = 0 <= dlo < cw
                                for h2 in range(2):
                                    sp = s_ps.tile([P, SCH], f32, name="sp", tag="s")
                                    for w0 in range(0, cw, 512):
                                        ww = min(512, cw - w0)
                                        diag_here = has_diag and w0 <= dlo < w0 + ww
                                        nc.tensor.matmul(
                                            sp[:, w0 : w0 + ww],
                                            lhsT=qT[_ts(h2, DH), pr, _ts(i, P)],
                                            rhs=kT[_ts(h2, DH), pr, c * SCH + w0 : c * SCH + w0 + ww],
                                            start=True,
                                            stop=not diag_here,
                                        )
                                        if diag_here:
                                            nc.tensor.matmul(
                                                sp[:, dlo : dlo + P],
                                                lhsT=ident,
                                                rhs=mask,
                                                start=False,
                                                stop=True,
                                            )
                                    sps2[h2].append((sp, cw))
                            for h2 in range(2):
                                sps = sps2[h2]
                                mx = statp.tile([P, 2], f32, tag="mx")
                                for c, (sp, cw) in enumerate(sps):
                                    nc.vector.reduce_max(
                                        mx[:, c : c + 1], sp[:, :cw], axis=AX.X
                                    )
                                negm = statp.tile([P, 1], f32, tag="negm")
                                if nch > 1:
                                    m1 = statp.tile([P, 1], f32, tag="m1")
                                    nc.vector.reduce_max(m1, mx[:, :nch], axis=AX.X)
                                    nc.vector.tensor_scalar_mul(negm, m1, -1.0)
                                else:
                                    nc.vector.tensor_scalar_mul(negm, mx[:, :1], -1.0)
                                prow = prowp.tile([P, L], bf16)
                                sums = statp.tile([P, 2], f32, tag="sums")
                                for c, (sp, cw) in enumerate(sps):
                                    nc.scalar.activation(
                                        prow[:, c * SCH : c * SCH + cw],
                                        sp[:, :cw],
                                        AF.Exp,
                                        bias=negm,
                                        accum_out=sums[:, c : c + 1],
                                    )
                                stot = statp.tile([P, 1], f32, tag="stot")
                                if nch > 1:
                                    nc.vector.reduce_sum(stot, sums[:, :nch], axis=AX.X)
                                else:
                                    nc.vector.tensor_copy(stot, sums[:, :1])
                                sinv = statp.tile([P, 1], f32, tag="sinv")
                                nc.vector.reciprocal(sinv, stot)
                                nc.vector.tensor_scalar_mul(
                                    prow[:, :klen], prow[:, :klen], sinv
                                )
                                nc.sync.dma_start_transpose(
                                    ptg[h2][:, : i + 1, s, :], prow[:, :klen]
                                )
                        # z matmuls for this group
                        for h2 in range(2):
                            hcol = (pr * 2 + h2) * DH
                            zps = z_ps.tile([DH, G * P], f32)
                            jmax = G * (g + 1)
                            for j in range(jmax):
                                sc = max(0, j - G * g)
                                nc.tensor.matmul(
                                    zps[:, sc * P :],
                                    lhsT=vv[:, j, hcol : hcol + DH],
                                    rhs=ptg[h2][:, j, sc:G, :],
                                    start=(j == 0),
                                    stop=(j == jmax - 1),
                                )
                            nc.scalar.copy(zst[g][_ts(h2, DH), pr, :], zps)

                # ---------------- Stage C: output projection ----------------
                for i in range(LT):
                    g, s = divmod(i, G)
                    osb = osbp.tile([P, DM], f32)
                    for mc2 in range(2):
                        ops = o_ps.tile([P, 512], f32)
                        for pr in range(NPAIR):
                            nc.tensor.matmul(
                                ops,
                                lhsT=r(zst[g][:, pr, _ts(s, P)], FP32R_OUT),
                                rhs=r(wo_t[:, pr, _ts(mc2, 512)], FP32R_OUT),
                                start=(pr == 0),
                                stop=(pr == 1),
                            )
                        nc.scalar.copy(osb[:, _ts(mc2, 512)], ops)
                    nc.sync.dma_start(out[_ts(i, P), :], osb)

    nc.finalize()
    return nc


def make_in_maps(normal_pre_resid, W_Q, W_K, W_V, W_O, b_Q, b_K, b_V, b_O):
    import ml_dtypes

    x = np.asarray(normal_pre_resid, np.float32)
    W_Q = np.asarray(W_Q, np.float32)
    W_K = np.asarray(W_K, np.float32)
    W_V = np.asarray(W_V, np.float32)
    W_O = np.asarray(W_O, np.float32)
    b_Q = np.asarray(b_Q, np.float32)
    b_K = np.asarray(b_K, np.float32)
    b_V = np.asarray(b_V, np.float32)

    mask = np.triu(np.full((P, P), NEG, np.float32), k=1).astype(ml_dtypes.bfloat16)
    ident = np.eye(P, dtype=np.float32).astype(ml_dtypes.bfloat16)
    in_maps = []
    for c in range(8):
        b, hg = divmod(c, 4)
        heads = [4 * hg + j for j in range(HPC)]
        xT = np.ascontiguousarray(x[b].T)  # [DM, L]

        def pack_qk(W, bias):
            prs = []
            for p_ in range(NPAIR):
                h0, h1 = heads[2 * p_], heads[2 * p_ + 1]
                wcat = np.concatenate([W[h0], W[h1]], axis=1)  # [DM, 128]
                bcat = np.concatenate([bias[h0], bias[h1]])[None, :]  # [1, 128]
                prs.append(np.concatenate([wcat, bcat], axis=0))  # [DM+1, 128]
            return np.ascontiguousarray(np.stack(prs))  # [2, DM+1, 128]

        wv_cat = np.concatenate([W_V[h] for h in heads], axis=1)  # [DM, 256]
        bv_cat = np.concatenate([b_V[h] for h in heads])[None, :]  # [1, 256]
        wv_full = np.ascontiguousarray(np.concatenate([wv_cat, bv_cat], axis=0))
        wo_prs = np.ascontiguousarray(
            np.stack(
                [
                    np.concatenate(
                        [W_O[heads[2 * p_]], W_O[heads[2 * p_ + 1]]], axis=0
                    )
                    for p_ in range(NPAIR)
                ]
            )
        )  # [2, 128, DM]

        in_maps.append(
            {
                "xT": xT,
                "wq": pack_qk(W_Q, b_Q),
                "wk": pack_qk(W_K, b_K),
                "wv": wv_full,
                "wo": wo_prs,
                "mask": mask,
                "ident": ident,
            }
        )
    return in_maps


def run_device(in_maps, **kwargs):
    from concourse.bass_utils import run_bass_kernel_spmd

    if "nc" not in _CACHE:
        _CACHE["nc"] = build_bass()
    return run_bass_kernel_spmd(_CACHE["nc"], in_maps, core_ids=list(range(8)), **kwargs)


def kernel(normal_pre_resid, W_Q, W_K, W_V, W_O, b_Q, b_K, b_V, b_O, **extra):
    b_O = np.asarray(b_O, np.float32)
    in_maps = make_in_maps(
        normal_pre_resid, W_Q, W_K, W_V, W_O, b_Q, b_K, b_V, b_O
    )
    res = run_device(in_maps)
    outs = [r["out"] for r in res.results]
    full = np.zeros((B, L, DM), np.float32)
    for c in range(8):
        full[c // 4] += outs[c]
    full += b_O[None, None, :]
    return full
